# revision 68
# baseline (speedup 1.0000x reference)
"""Trainium2 Bass kernel for nn_Attention_41678362640976.

ViT-style attention block with a CLS-row prior injection:
  LayerNorm -> QKV (no bias) -> per-head S = q k^T * d^-0.5
  -> CLS row replaced by softmax(S[0,1:]) + canny_prior + noise_prior
  -> full softmax -> attn @ v -> out proj (+bias).

Sharding: pure data-parallel over batch, one batch element per NeuronCore
(B == 8 == n_cores). Each core runs an identical single-core program.

Per-core dataflow (N=1025 tokens, D=768, H=12 heads, HD=64):
  A. LayerNorm on x tiles [128,768] (bn_stats/bn_aggr), normalize
     (x-mu)*rstd in one tensor_scalar, PE-transpose, ln scale/bias applied
     on the PSUM->SBUF copies producing xnT (bf16, for v) and xnT8 (fp8,
     for q/k).
  B. v natural layout via xnT-stationary bf16 matmuls -> vsb
     [128, 9, 12, 65] with a ones column (AV then also emits softmax
     denominators).  q/k projections as fp8 DoubleRow matmuls (contraction
     256/instr); psum copied to qk8flat fp8, bounced through DRAM into the
     DoubleRow-interleaved layout qk8i [32, 2(par), 2(qk), 2(t), 1040].
  C. S^T per head/j-tile: one DoubleRow matmul pair -> psum [128,1024],
     exp on ACT (scale=D^-0.5 folded in) -> Et bf16.  CLS row, E-last row
     and the i=1024 column computed via block-diagonal fp8 matmuls; CLS
     softmax+priors patched into Et column 0 / elast column 0.
  D. AV flipped: out[i-tile, 65] = Et^T @ vsb accumulated over j; ones
     column gives per-partition denominators; normalize on the PSUM->SBUF
     copy (tensor_scalar by recip column) -> Obf bf16; PE-transpose to
     OsbT [128, 6, 1025].
  E. out = OsbT^T @ w_out (bf16) + b_out, DMA out.
"""

import numpy as np

import concourse.bass as bass
import concourse.mybir as mybir
import bass_rust as _bass_rust
from concourse.tile import TileContext
from concourse.bass_utils import run_bass_kernel_spmd, ml_dtypes

P = 128
N = 1025          # tokens (CLS + 32*32 patches)
NP = 1040         # N padded to a multiple of 16 (DoubleRow weight stride)
D = 768
H = 12
HD = 64
KT = 6            # contraction tiles of 128 over D
NT = 8            # full 128-token tiles; token 1024 handled separately
SCALE = float(D) ** -0.5
EPS = 1e-5
F32 = mybir.dt.float32
F32R = mybir.dt.float32r
BF16 = mybir.dt.bfloat16
FP8 = mybir.dt.float8e4
AF = mybir.ActivationFunctionType
ALU = mybir.AluOpType
DR = mybir.MatmulPerfMode.DoubleRow

# i-chunks for PSUM-bank-sized matmul outputs
CH2 = [(0, 512), (512, 512)]
CH3 = CH2 + [(1024, 1)]

NPAIR = 6         # head pairs


def build_core_program():
    nc = bass.Bass()

    x_d = nc.dram_tensor("x", [N, D], BF16, kind="ExternalInput")
    canny_d = nc.dram_tensor("canny", [1, 32, 32], F32, kind="ExternalInput")
    noise_d = nc.dram_tensor("noise", [32, 32], F32, kind="ExternalInput")
    cqk_d = nc.dram_tensor("cqk", [P, H], F32, kind="ExternalInput")
    cv_d = nc.dram_tensor("cv", [D], BF16, kind="ExternalInput")
    wqk8_d = nc.dram_tensor("wqk8", [P, H * 3 * 2 * P], FP8, kind="ExternalInput")
    wv_d = nc.dram_tensor("wv", [P, KT * D], BF16, kind="ExternalInput")
    wout_d = nc.dram_tensor("wout", [P, KT * D], BF16, kind="ExternalInput")
    bout_d = nc.dram_tensor("b_out", [D], BF16, kind="ExternalInput")
    out_d = nc.dram_tensor("out", [N, D], F32, kind="ExternalOutput")

    with TileContext(nc) as tc:
        with (
            tc.tile_pool(name="persist", bufs=1) as pp,
            tc.tile_pool(name="once", bufs=1) as op,
            tc.tile_pool(name="work", bufs=2) as wp,
            tc.tile_pool(name="xtp", bufs=4) as xtp,
            tc.tile_pool(name="bigscratch", bufs=1) as bsp,
            tc.tile_pool(name="bigscratch2", bufs=1) as bs2,
            tc.tile_pool(name="ebuf", bufs=4) as ep,
            tc.tile_pool(name="elp", bufs=2) as elp,
            tc.tile_pool(name="obfp", bufs=2) as obfp,
            tc.tile_pool(name="dram", bufs=1, space="DRAM") as dp,
            tc.tile_pool(name="ps_sm", bufs=2, space="PSUM") as ps_sm,
            tc.tile_pool(name="ps_big", bufs=2, space="PSUM") as ps_big,
            tc.tile_pool(name="ps_ss", bufs=2, space="PSUM") as ps_ss,
        ):
            # ---------------- persistent tiles ----------------
            # xnT is dead after the v projection; acc (out-proj staging)
            # reuses its space via the bufs=1 bigscratch pool
            xnT = bsp.tile([P, KT, N], BF16, name="xnT")
            xnT8 = pp.tile([P, KT, NP], FP8, name="xnT8")
            qk8flat = pp.tile([P, H, NP], FP8, name="qk8flat")
            vsb = pp.tile([P, NT + 1, H, HD + 1], BF16, name="vsb")
            OsbT = pp.tile([P, KT, N], BF16, name="OsbT")
            # wvall is dead after the v projection; the per-pair obf tiles
            # reuse its space via the bufs=1 bigscratch2 pool
            wvall = bs2.tile([P, KT, 2, 384], BF16, name="wvall")
            wqk8 = pp.tile([P, H, 3, 2, P], FP8, name="wqk8")
            wout_sb = pp.tile([P, KT, D], BF16, name="wout_sb")
            brep = pp.tile([P, D], BF16, name="brep")
            cqk_col = pp.tile([P, H], F32, name="cqk_col")
            cvrep = pp.tile([P, 2, 384], BF16, name="cvrep")
            # three DoubleRow-interleaved q/k pair buffers packed at
            # partition bases 0/32/64 (matmul base-partition legal set)
            qk8i3 = pp.tile([96, 2, 2, 2, 1024], FP8, name="qk8i3")
            id128 = pp.tile([P, P], BF16, name="id128")
            id12 = pp.tile([H, H], BF16, name="id12")
            # padded to 16 columns: DoubleRow weight APs want 16-aligned strides
            q0b8 = pp.tile([P, KT, 16], FP8, name="q0b8")
            k1024b8 = pp.tile([P, KT, 16], FP8, name="k1024b8")
            q1024b8 = pp.tile([P, KT, 16], FP8, name="q1024b8")
            # f32 scratch rows packed at 32-aligned partition offsets
            f32pk = pp.tile([P, N], F32, name="f32pk")
            clsrow = f32pk[0:H, :]
            e1row = f32pk[32 : 32 + H, 0 : N - 1]
            crow = f32pk[64:65, 0 : N - 1]
            nrow = f32pk[96:97, 0 : N - 1]
            # cnrep reuses the crow rows once the priors are staged to DRAM
            cnrep = f32pk[64 : 64 + H, 0 : N - 1]
            bf16pk = pp.tile([64, N], BF16, name="bf16pk")
            expu = bf16pk[0:H, :]
            elast = bf16pk[32 : 32 + H, :]
            sum1 = pp.tile([H, 1], F32, name="sum1")
            recip1 = pp.tile([H, 1], F32, name="recip1")
            expUc = pp.tile([P, NT + 1, H], BF16, name="expUc")
            e1024 = pp.tile([P, NT, H], BF16, name="e1024")
            eps_col = pp.tile([P, 1], F32, name="eps_col")

            # DRAM scratch
            scr_cn = dp.tile([1, N - 1], F32, name="scr_cn")
            scr_el = dp.tile([H, N], BF16, name="scr_el")

            # ---------------- constants + weight loads ----------------
            from concourse.masks import make_identity
            nc.vector.memset(id128[:], 0.0)
            make_identity(nc, id128[:], nomemset=True)
            nc.vector.memset(id12[:], 0.0)
            make_identity(nc, id12[:], nomemset=True)
            nc.vector.memset(eps_col[:], EPS)
            warm = op.tile([1, 1], F32, name="warm")
            nc.scalar.activation(warm[:], eps_col[0:1, :], AF.Exp)
            nc.vector.memset(vsb[:, :, :, HD : HD + 1], 1.0)
            # zero the padding columns of qk8flat (the DRAM bounce reads them)
            nc.vector.memset(qk8flat[:, :, N:NP], 0.0)

            nc.gpsimd.dma_start(cqk_col[:], cqk_d[:])
            nc.gpsimd.dma_start(
                cvrep[:].rearrange("p a b -> p (a b)"),
                cv_d[None, :].to_broadcast((P, D)),
            )

            # ---------------- priors (early; Pool is idle here) ----------
            csum = op.tile([1, 1], F32, name="csum")
            nsum = op.tile([1, 1], F32, name="nsum")
            crcp = op.tile([1, 1], F32, name="crcp")
            nrcp = op.tile([1, 1], F32, name="nrcp")
            nc.gpsimd.dma_start(crow[:], canny_d[:].rearrange("a b c -> a (b c)"))
            nc.gpsimd.dma_start(nrow[:], noise_d[:].rearrange("b c -> (b c)")[None, :])
            nc.scalar.activation(crow[:], crow[:], AF.Identity, accum_out=csum[:])
            nc.scalar.activation(nrow[:], nrow[:], AF.Identity, accum_out=nsum[:])
            nc.vector.tensor_scalar_add(csum[:], csum[:], float(N - 1))
            nc.vector.reciprocal(crcp[:], csum[:])
            nc.vector.reciprocal(nrcp[:], nsum[:])
            nc.vector.tensor_scalar(
                crow[:], crow[:], 1.0, crcp[:, 0:1], ALU.add, ALU.mult
            )
            nc.vector.tensor_scalar_mul(nrow[:], nrow[:], nrcp[:, 0:1])
            nc.vector.tensor_add(crow[:], crow[:], nrow[:])
            nc.gpsimd.dma_start(scr_cn[:], crow[:])
            nc.gpsimd.dma_start(cnrep[:], scr_cn[:].to_broadcast((H, N - 1)))

            # ---------------- priors (independent, early) ----------------
            # ---------------- A: LayerNorm + transpose ----------------
            for tt in range(NT + 1):
                rows = P if tt < NT else 1
                xt = xtp.tile([P, D], BF16, name="xt16")
                nc.sync.dma_start(xt[:rows], x_d[tt * P : tt * P + rows, :])
                if tt == 2:
                    # q/k weights early: they gate the first projections
                    nc.sync.dma_start(
                        wqk8[:],
                        wqk8_d[:].rearrange(
                            "p (h a b c) -> p h a b c", h=H, a=3, b=2
                        ),
                    )
                if tt == 6:
                    nc.sync.dma_start(
                        wvall[:],
                        wv_d[:].rearrange("p (k c f) -> p k c f", k=KT, c=2),
                    )
                stats = wp.tile([P, 2, 6], F32, name="stats")
                mv = wp.tile([P, 2], F32, name="mv")
                nc.vector.bn_stats(stats[:rows, 0, :], xt[:rows, 0 : D // 2])
                nc.vector.bn_stats(stats[:rows, 1, :], xt[:rows, D // 2 : D])
                nc.vector.bn_aggr(mv[:rows], stats[:rows])
                lnv = wp.tile([P, 1], F32, name="lnv")
                rstd = wp.tile([P, 1], F32, name="rstd")
                nc.scalar.activation(
                    lnv[:rows], mv[:rows, 1:2], AF.Ln, bias=eps_col[:rows, 0:1]
                )
                nc.scalar.activation(rstd[:rows], lnv[:rows], AF.Exp, scale=-0.5)
                xc = wp.tile([P, D], BF16, name="xc")
                nc.vector.tensor_scalar(
                    xc[:rows],
                    xt[:rows],
                    mv[:rows, 0:1],
                    rstd[:rows, 0:1],
                    ALU.subtract,
                    ALU.mult,
                )
                for kt in range(KT):
                    pst = ps_sm.tile([P, P], BF16, name="pst", tag="ps_small")
                    nc.tensor.transpose(
                        pst[:, :rows],
                        xc[:rows, kt * P : (kt + 1) * P],
                        id128[:rows, :rows],
                    )
                    if kt % 3 == 1:
                        nc.scalar.copy(
                            xnT[:, kt, tt * P : tt * P + rows], pst[:, :rows]
                        )
                    else:
                        xeng = nc.vector if kt % 3 == 0 else nc.gpsimd
                        xeng.tensor_copy(
                            xnT[:, kt, tt * P : tt * P + rows], pst[:, :rows]
                        )
                    nc.gpsimd.tensor_copy(
                        xnT8[:, kt, tt * P : tt * P + rows], pst[:, :rows]
                    )

            # ---------------- B2: v in natural layout ----------------
            for tt in range(NT + 1):
                rows = P if tt < NT else 1
                for c2 in range(2):
                    pb = ps_big.tile([P, 512], F32, name="pb", tag="ps_big")
                    for kt in range(KT):
                        nc.tensor.matmul(
                            pb[:rows, :384],
                            xnT[:, kt, tt * P : tt * P + rows],
                            wvall[:, kt, c2, :],
                            start=(kt == 0),
                            stop=(kt == KT - 1),
                        )
                    nc.vector.tensor_tensor(
                        vsb[:rows, tt, 6 * c2 : 6 * c2 + 6, 0:HD],
                        pb[:rows, :384].rearrange("p (h f) -> p h f", h=6),
                        cvrep[:rows, c2, :].rearrange("p (h f) -> p h f", h=6),
                        ALU.add,
                    )

            # ---------------- B: q,k DoubleRow projections ----------------
            Et_by_head = {}
            qk8i_by_pair = {}

            def emit_st_pair(dt_):
                # S^T + exp for the two heads of pair dt_
                qk8i = qk8i_by_pair[dt_]
                for par in (0, 1):
                    h = 2 * dt_ + par
                    Et = ep.tile([P, NT, 1024], BF16, name="Et", tag="Etc")
                    Et_by_head[h] = Et
                    for jt in range(NT):
                        pss = ps_ss.tile([P, 1024], F32, name="pss", tag="pss")
                        for cs, cl in CH2:
                            nc.tensor.matmul(
                                pss[:, cs : cs + cl],
                                qk8i[:, par, 1, :, jt * P : (jt + 1) * P],
                                qk8i[:, par, 0, :, cs : cs + cl],
                                start=True,
                                stop=True,
                                perf_mode=DR,
                            )
                        nc.scalar.activation(
                            Et[:, jt, :], pss[:], AF.Exp, scale=SCALE
                        )

            for dt_ in range(KT):
                for mi, mt in enumerate((dt_, 6 + dt_)):
                    for ci, (cs, cl) in enumerate(CH3):
                        pb = ps_big.tile([P, 512], F32, name="pb", tag="ps_big")
                        for kt2 in range(3):
                            nc.tensor.matmul(
                                pb[:, :cl],
                                wqk8[:, mt, kt2, :, :],
                                xnT8[:, 2 * kt2 : 2 * kt2 + 2, cs : cs + cl],
                                start=(kt2 == 0),
                                stop=(kt2 == 2),
                                perf_mode=DR,
                            )
                        eng = nc.vector if (mi + ci) % 2 == 0 else nc.gpsimd
                        eng.tensor_scalar_add(
                            qk8flat[:, mt, cs : cs + cl], pb[:, :cl],
                            cqk_col[:, mt : mt + 1],
                        )
                # partition-shifted sbuf->sbuf DMAs into the DoubleRow-
                # interleaved layout: qk8i[p, par, a, t, i] = qk[64par+32t+p]
                # (i=1024 column not needed: the e1024/elast paths cover it)
                qk8i = qk8i3[32 * (dt_ % 3) : 32 * (dt_ % 3) + 32]
                qk8i_by_pair[dt_] = qk8i
                qkv_view = qk8flat[:].rearrange("p (a h) i -> p a h i", a=2)
                for par in (0, 1):
                    for t in (0, 1):
                        base = 64 * par + 32 * t
                        nc.sync.dma_start(
                            qk8i[:, par, :, t, :],
                            qkv_view[base : base + 32, :, dt_, 0:1024],
                        )
                # S^T of pair 0 only: the cls pipeline (which gates AV)
                # must get its ACT-queue slot before the later exp streams
                if dt_ == 1:
                    emit_st_pair(0)

            # late weight loads (needed from the first out-proj half on)
            nc.gpsimd.dma_start(
                wout_sb[:], wout_d[:].rearrange("p (k e) -> p k e", k=KT)
            )
            nc.gpsimd.dma_start(brep[:], bout_d[None, :].to_broadcast((P, D)))

            # ---------------- C2: CLS row + last-token helpers ----------------
            for blk, srcslot, col in (
                (q0b8, 0, 0),
                (k1024b8, 1, 1024),
                (q1024b8, 0, 1024),
            ):
                nc.vector.memset(blk[:], 0.0)
                for h in range(H):
                    qb = (h % 2) * HD
                    nc.vector.tensor_copy(
                        blk[qb : qb + HD, h // 2, h : h + 1],
                        qk8flat[qb : qb + HD, 6 * srcslot + h // 2, col : col + 1],
                    )

            # cls logits row for every head: [12, 1025] (raw, scaled on copy)
            for cs, cl in CH3:
                pc = ps_big.tile([P, 512], F32, name="pc", tag="ps_big")
                for kt2 in range(3):
                    nc.tensor.matmul(
                        pc[0:16, :cl],
                        q0b8[:, 2 * kt2 : 2 * kt2 + 2, :],
                        qk8flat[:, 6 + 2 * kt2 : 8 + 2 * kt2, cs : cs + cl],
                        start=(kt2 == 0),
                        stop=(kt2 == 2),
                        perf_mode=DR,
                    )
                nc.vector.tensor_scalar_mul(clsrow[:, cs : cs + cl], pc[0:H, :cl], SCALE)

            # E_last = exp(S^T[1024, :]) for every head (row j=1024)
            for cs, cl in CH3:
                pc = ps_big.tile([P, 512], F32, name="pc", tag="ps_big")
                for kt2 in range(3):
                    nc.tensor.matmul(
                        pc[0:16, :cl],
                        k1024b8[:, 2 * kt2 : 2 * kt2 + 2, :],
                        qk8flat[:, 2 * kt2 : 2 + 2 * kt2, cs : cs + cl],
                        start=(kt2 == 0),
                        stop=(kt2 == 2),
                        perf_mode=DR,
                    )
                nc.scalar.activation(
                    elast[:, cs : cs + cl], pc[0:H, :cl], AF.Exp, scale=SCALE
                )

            # E_1024 column (i=1024, j<1024) for every head: [128, 8, 12]
            for jt in range(NT):
                pc = ps_sm.tile([P, 16], F32, name="pe", tag="ps_small")
                for kt2 in range(3):
                    nc.tensor.matmul(
                        pc[:],
                        qk8flat[:, 6 + 2 * kt2 : 8 + 2 * kt2, jt * P : (jt + 1) * P],
                        q1024b8[:, 2 * kt2 : 2 * kt2 + 2, :],
                        start=(kt2 == 0),
                        stop=(kt2 == 2),
                        perf_mode=DR,
                    )
                nc.scalar.activation(e1024[:, jt, :], pc[:, 0:H], AF.Exp, scale=SCALE)

            # first softmax over cls row cols 1..1024, plus priors
            nc.scalar.activation(e1row[:], clsrow[:, 1:N], AF.Exp, accum_out=sum1[:])
            nc.vector.reciprocal(recip1[:], sum1[:])
            nc.vector.tensor_scalar_mul(clsrow[:, 1:N], e1row[:], recip1[:, 0:1])
            nc.vector.tensor_add(clsrow[:, 1:N], clsrow[:, 1:N], cnrep[:])
            # exp of the patched row (col 0 keeps the original logit)
            nc.scalar.activation(expu[:], clsrow[:], AF.Exp)
            # patch elast col 0 with expu col 1024, then stage to DRAM
            nc.vector.tensor_copy(elast[:, 0:1], expu[:, 1024:1025])
            nc.gpsimd.dma_start(scr_el[:], elast[:])
            # transpose expu rows into columns [128, 9, 12]
            for jt in range(NT + 1):
                rows = P if jt < NT else 1
                pst = ps_sm.tile([P, H], BF16, name="pu", tag="ps_small")
                nc.tensor.transpose(
                    pst[:rows, :], expu[:, jt * P : jt * P + rows], id12[:]
                )
                nc.vector.tensor_copy(expUc[:rows, jt, :], pst[:rows, :])

            for _d in range(1, KT):
                emit_st_pair(_d)

            # ---------------- F (half): out projection over a pair subset ---
            acc = bsp.tile([P, NT + 1, D], BF16, name="acc")

            def emit_outproj_half(pairs, stage, tts=None):
                # stage 0: acc = po + bias; 1: acc += po; 2: ot = po + acc, DMA
                for tt in (range(NT + 1) if tts is None else tts):
                    rows = P if tt < NT else 1
                    # reuses the xt name slot (same shape/dtype, disjoint phase)
                    ot = wp.tile([P, D], F32, name="xt") if stage == 2 else None
                    for c2 in range(2):
                        po = ps_sm.tile([P, 384], F32, name="po", tag="ps_small")
                        # latest pair first so the psum group opens on its
                        # latest dependency
                        order = [pairs[-1]] + list(pairs[:-1])
                        for i, pg in enumerate(order):
                            nc.tensor.matmul(
                                po[:rows, :384],
                                OsbT[:, pg, tt * P : tt * P + rows],
                                wout_sb[:, pg, c2 * 384 : (c2 + 1) * 384],
                                start=(i == 0),
                                stop=(i == len(pairs) - 1),
                            )
                        sl = slice(c2 * 384, (c2 + 1) * 384)
                        eng = nc.vector if c2 == 0 else nc.gpsimd
                        if stage == 0:
                            eng.tensor_add(
                                acc[:rows, tt, sl], po[:rows, :384], brep[:rows, sl]
                            )
                        elif stage == 1:
                            eng.tensor_add(
                                acc[:rows, tt, sl], po[:rows, :384],
                                acc[:rows, tt, sl],
                            )
                        else:
                            eng.tensor_add(
                                ot[:rows, sl], po[:rows, :384], acc[:rows, tt, sl]
                            )
                    if stage == 2:
                        nc.sync.dma_start(
                            out_d[tt * P : tt * P + rows, :], ot[:rows]
                        )

            # ---------------- D: flipped AV per head pair ----------------
            for pg in range(NPAIR):
                el_of = {}
                for par in (0, 1):
                    h = 2 * pg + par
                    # patch Et column 0 with the corrected CLS column
                    nc.vector.tensor_copy(
                        Et_by_head[h][:, :, 0:1].rearrange("p a b -> p (a b)"),
                        expUc[:, 0:NT, h],
                    )
                    el_h = elp.tile([1, N], BF16, name="el_h", tag="el_h")
                    el_of[h] = el_h
                    nc.gpsimd.dma_start(el_h[:], scr_el[h : h + 1, :])
                obf = obfp.tile([P, NT + 1, P], BF16, name="obf", tag="obf")
                for it in range(NT + 1):
                    rows = P if it < NT else 1
                    pav = ps_big.tile([P, 512], F32, name="pav", tag="ps_big")
                    for par in (0, 1):
                        h = 2 * pg + par
                        Et = Et_by_head[h]
                        co = par * (HD + 1)
                        for jt in range(NT):
                            if it < NT:
                                lhsT = Et[:, jt, it * P : it * P + rows]
                            else:
                                lhsT = e1024[:, jt, h : h + 1]
                            nc.tensor.matmul(
                                pav[:rows, co : co + HD + 1],
                                lhsT,
                                vsb[:, jt, h, :],
                                start=(jt == 0),
                                stop=False,
                            )
                        nc.tensor.matmul(
                            pav[:rows, co : co + HD + 1],
                            el_of[h][0:1, it * P : it * P + rows],
                            vsb[0:1, NT, h, :],
                            start=False,
                            stop=True,
                        )
                    # denominators -> reciprocals (both heads at once)
                    rcp = wp.tile([P, 2], F32, name="rcpw")
                    nc.vector.reciprocal(
                        rcp[:rows, :],
                        pav[:rows, 0 : 2 * (HD + 1)].rearrange(
                            "p (a c) -> p a c", a=2
                        )[:, :, HD],
                    )
                    # normalize on the PSUM->SBUF copy
                    for par in (0, 1):
                        co = par * (HD + 1)
                        nc.vector.tensor_scalar_mul(
                            obf[:rows, it, par * HD : par * HD + HD],
                            pav[:rows, co : co + HD],
                            rcp[:rows, par : par + 1],
                        )
                # transpose Obf -> OsbT
                for it in range(NT + 1):
                    rows = P if it < NT else 1
                    pto = ps_sm.tile([P, P], BF16, name="pto", tag="ps_small")
                    nc.tensor.transpose(
                        pto[:, :rows], obf[:rows, it, :], id128[:rows, :rows]
                    )
                    nc.vector.tensor_copy(
                        OsbT[:, pg, it * P : it * P + rows], pto[:, :rows]
                    )
                if pg in (2, 3):
                    emit_outproj_half(
                        (0, 1, 2), stage=0,
                        tts=range(5 * (pg - 2), min(5 * (pg - 1), NT + 1)),
                    )
                if pg == 4:
                    emit_outproj_half((3, 4), stage=1)

            # ---------------- F: out projection (second half) ----------------
            emit_outproj_half((5,), stage=2)

    _bass_rust.generate_event_semaphores(nc)
    return nc


def _prep_weights(ln_w, ln_b, w_qkv, w_out, b_out):
    """Host-side weight relayouts and the standard LayerNorm-affine fold:
    x_norm @ W = ((x-mu)*rstd) @ (diag(ln_w) W) + ln_b @ W."""
    f8 = ml_dtypes.float8_e4m3
    bf = ml_dtypes.bfloat16
    wqkv = np.asarray(w_qkv, dtype=np.float32)
    lnw = np.asarray(ln_w, dtype=np.float32)
    lnb = np.asarray(ln_b, dtype=np.float32)
    wqkv_s = wqkv * lnw[:, None]
    c = lnb @ wqkv  # [2304] additive term from the ln bias
    # wqk8[p, mt, kt2, t, m] = wqkv_s[(2kt2+t)*128+p, mt*128+m]
    w8 = wqkv_s[:, 0 : 2 * D].astype(f8).reshape(3, 2, P, H, P)
    w8 = np.ascontiguousarray(np.transpose(w8, (2, 3, 0, 1, 4))).reshape(P, -1)
    # wv[p, kt, c2, f] = wqkv_s[kt*128+p, 1536 + c2*384 + f]
    wv = wqkv_s[:, 2 * D : 3 * D].astype(bf)
    wv = np.ascontiguousarray(
        np.transpose(wv.reshape(KT, P, 2, 384), (1, 0, 2, 3))
    ).reshape(P, -1)
    # wout[p, kt, e] = w_out[kt*128+p, e]
    wo = np.asarray(w_out, dtype=np.float32).astype(bf)
    wo = np.ascontiguousarray(
        np.transpose(wo.reshape(KT, P, D), (1, 0, 2))
    ).reshape(P, -1)
    cqk = np.ascontiguousarray(c[0 : 2 * D].reshape(H, P).T)
    return {
        "wqk8": w8,
        "wv": wv,
        "wout": wo,
        "cqk": cqk,
        "cv": np.ascontiguousarray(c[2 * D : 3 * D].astype(bf)),
        "b_out": np.ascontiguousarray(
            np.asarray(b_out, dtype=np.float32).astype(bf)
        ),
    }


_NC_CACHE = None


def kernel(**inputs) -> np.ndarray:
    global _NC_CACHE
    x = np.ascontiguousarray(
        np.asarray(inputs["x"], dtype=np.float32).astype(ml_dtypes.bfloat16)
    )
    canny = np.ascontiguousarray(np.asarray(inputs["canny"], dtype=np.float32))
    noise = np.ascontiguousarray(np.asarray(inputs["noise"], dtype=np.float32))
    shared = _prep_weights(
        inputs["ln_w"], inputs["ln_b"], inputs["w_qkv"],
        inputs["w_out"], inputs["b_out"],
    )

    B = x.shape[0]
    assert B == 8, f"expected batch 8, got {B}"

    if _NC_CACHE is None:
        _NC_CACHE = build_core_program()
    nc = _NC_CACHE

    in_maps = [
        {"x": x[b], "canny": canny[b], "noise": noise[b], **shared}
        for b in range(B)
    ]
    res = run_bass_kernel_spmd(nc, in_maps, core_ids=list(range(B)))
    out = np.stack([res.results[b]["out"] for b in range(B)], axis=0)
    return out.astype(np.float32)
